# revision 1
# baseline (speedup 1.0000x reference)
"""Trainium2 Bass kernel for nn_DiffusionModel1d (batched 1-D diffusion solve).

Math: the reference solves A(K) u = f per batch row with K = exp(x) via the
Thomas algorithm, where A = G^T diag(K_hat) G, G the n x n lower-bidiagonal
difference matrix (1 on diag, -1 on subdiag) and
K_hat = (2*K_0, K_1, ..., K_{n-1}).  Hence

    u = h2 * G^{-1} diag(K_hat)^{-1} G^{-T} f
      = h2 * cumsum_j( w_j * exp(-x_j) ),   w = suffix_sum(f), w_0 halved.

So the whole solve is: one exp, one elementwise multiply by a shared
per-column vector, and one hardware prefix-sum scan along the grid dim.
Pure data parallel over batch: 8192 rows -> 1024 rows per core x 8 cores.
The tiny shared w vector (2047 elements, derived from the replicated f_rhs
by one suffix-sum pass) is prepared host-side and shipped replicated across
the 128 SBUF partitions, so the device pipeline has no serial prologue.

Engine budget per core (measured): DVE 2x bf16 mult 1.2us + 2 chained fp32
scans 2x2.3us per 128-row group (x8 groups), ACT exp ~1.5us, DMA 16.8 MB.
"""

import os
import sys

import numpy as np

sys.path.insert(0, "/opt/trn_rl_repo")

import ml_dtypes

import concourse.bacc as bacc
import concourse.mybir as mybir
import concourse.tile as tile
from concourse import bass_utils

B, M = 8192, 2048
N = M - 1
NCORES = 8
BC = B // NCORES          # 1024 batch rows per core
P = 128                   # SBUF partitions
GROUPS = BC // P          # 8 partition-groups per core
H2 = (1.0 / N) ** 2

_cached_nc = None
LAST_RESULTS = None


def _build_kernel():
    fp32 = mybir.dt.float32
    bf16 = mybir.dt.bfloat16
    nc = bacc.Bacc(
        "TRN2",
        target_bir_lowering=False,
        debug=False,
        enable_asserts=False,
        num_devices=NCORES,
    )
    x_d = nc.dram_tensor("x", (BC, M), fp32, kind="ExternalInput").ap()
    w_d = nc.dram_tensor("w", (P, N), bf16, kind="ExternalInput").ap()
    o_d = nc.dram_tensor("out", (BC, N), fp32, kind="ExternalOutput").ap()

    add = mybir.AluOpType.add
    bypass = mybir.AluOpType.bypass

    with tile.TileContext(nc) as tc:
        with (
            tc.tile_pool(name="const", bufs=1) as cpool,
            tc.tile_pool(name="xin", bufs=GROUPS) as xpool,
            tc.tile_pool(name="work", bufs=4) as pool,
        ):
            # shared per-column weights, already broadcast across partitions;
            # issue on ACT's HWDGE queue so it is not behind the x DMAs
            wb = cpool.tile([P, N], bf16, tag="wb")
            nc.scalar.dma_start(out=wb, in_=w_d)

            # hoist all input loads: they have no dependencies, and the sync
            # DMA stream is in-order — emitting them first keeps later
            # groups' loads from queueing behind output dispatches.
            # Group 0's load is split so its exp can start half a transfer
            # earlier (shorter ramp).
            half = 1024
            xts = []
            for g in range(GROUPS):
                xt = xpool.tile([P, M], fp32, tag="x")
                if g == 0:
                    nc.sync.dma_start(out=xt[:, :half], in_=x_d[:P, :half])
                    nc.sync.dma_start(out=xt[:, half:], in_=x_d[:P, half:])
                else:
                    nc.sync.dma_start(out=xt, in_=x_d[g * P : (g + 1) * P, :])
                xts.append(xt)

            # ---- per-group pipeline: exp(-x) -> *w -> cumsum -> DMA out.
            # Group 0's exp/mult run in column halves (shorter ramp); the
            # first and last groups' scan/store run in chained halves
            # (shorter ramp and tail).
            for g in range(GROUPS):
                rows = slice(g * P, (g + 1) * P)
                xt = xts[g]
                et = pool.tile([P, N], bf16, tag="e")
                vt = pool.tile([P, N], bf16, tag="v")
                ut = pool.tile([P, N], fp32, tag="u")
                em_splits = [(0, half), (half, N)] if g == 0 else [(0, N)]
                for c0, c1 in em_splits:
                    nc.scalar.activation(
                        out=et[:, c0:c1],
                        in_=xt[:, c0:c1],
                        func=mybir.ActivationFunctionType.Exp,
                        scale=-1.0,
                    )
                    nc.vector.tensor_mul(
                        out=vt[:, c0:c1], in0=et[:, c0:c1], in1=wb[:, c0:c1]
                    )
                su_splits = (
                    [(0, half), (half, N)]
                    if g in (0, GROUPS - 2, GROUPS - 1)
                    else [(0, N)]
                )
                for si, (c0, c1) in enumerate(su_splits):
                    # paired scan: one scan step absorbs TWO elements
                    # (state = (v[2t] + state) + v[2t+1]), so the 2 cyc/elem
                    # scan covers only the odd positions; evens follow with
                    # a 1 cyc/elem shifted add.  ~20% less DVE per group.
                    npairs = (c1 - c0) // 2
                    nc.vector.tensor_tensor_scan(
                        out=ut[:, c0 + 1 : c0 + 2 * npairs : 2],
                        data0=vt[:, c0 : c0 + 2 * npairs : 2],
                        data1=vt[:, c0 + 1 : c0 + 2 * npairs : 2],
                        initial=0.0 if si == 0 else ut[:, c0 - 1 : c0],
                        op0=add,
                        op1=add,
                    )
                    if si == 0:
                        # u[0] = v[0] (cast bf16->fp32 off the DVE)
                        nc.scalar.copy(out=ut[:, 0:1], in_=vt[:, 0:1])
                        ev0 = c0 + 2
                    else:
                        # chained range: u[c0] = u[c0-1] + v[c0] folds into
                        # the evens add below
                        ev0 = c0
                    nc.vector.tensor_add(
                        out=ut[:, ev0:c1:2],
                        in0=ut[:, ev0 - 1 : c1 - 1 : 2],
                        in1=vt[:, ev0:c1:2],
                    )
                    nc.sync.dma_start(out=o_d[rows, c0:c1], in_=ut[:, c0:c1])

    nc.compile()
    return nc


def _get_nc():
    global _cached_nc
    if _cached_nc is None:
        _cached_nc = _build_kernel()
    return _cached_nc


def _make_w(f_rhs: np.ndarray) -> np.ndarray:
    """w = h2 * suffix_sum(f), w[0] halved; replicated to [P, N] bf16."""
    w = np.cumsum(f_rhs[::-1].astype(np.float64))[::-1] * H2
    w[0] *= 0.5
    wrow = w.astype(ml_dtypes.bfloat16)
    return np.ascontiguousarray(np.broadcast_to(wrow[None, :], (P, N)))


def kernel(x: np.ndarray, f_rhs: np.ndarray) -> np.ndarray:
    assert x.shape == (B, M) and f_rhs.shape == (N,)
    x = np.ascontiguousarray(x, dtype=np.float32)
    wb = _make_w(np.asarray(f_rhs, dtype=np.float32))
    nc = _get_nc()
    in_maps = [
        {"x": x[c * BC : (c + 1) * BC], "w": wb} for c in range(NCORES)
    ]
    res = bass_utils.run_bass_kernel_spmd(
        nc,
        in_maps,
        core_ids=list(range(NCORES)),
        trace=bool(int(os.environ.get("KERNEL_TRACE", "0"))),
    )
    global LAST_RESULTS
    LAST_RESULTS = res
    out = np.concatenate(
        [res.results[c]["out"] for c in range(NCORES)], axis=0
    ).astype(np.float32)
    return out



# revision 8
# speedup vs baseline: 1.5278x; 1.5278x over previous
"""Trainium2 Bass kernel for nn_DiffusionModel1d (batched 1-D diffusion solve).

Math: the reference solves A(K) u = f per batch row with K = exp(x) via the
Thomas algorithm, where A = G^T diag(K_hat) G, G the n x n lower-bidiagonal
difference matrix (1 on diag, -1 on subdiag) and
K_hat = (2*K_0, K_1, ..., K_{n-1}).  Hence

    u = h2 * G^{-1} diag(K_hat)^{-1} G^{-T} f
      = h2 * cumsum_j( w_j * exp(-x_j) ),   w = suffix_sum(f), w_0 halved.

Layout: TRANSPOSED (grid dim in partitions).  The prefix sum along the grid
dim becomes a per-chunk triangular matrix multiply on the Tensor engine
(lhsT[k, j] = w'_k for k <= j), which removes the Vector-engine scan that
bottlenecked the batch-major version.  Per core (1024 batch cols):

  - 16 grid chunks of 128; e = exp(-x) on ACT (fp16), waves of 2 chunks.
  - main matmul per chunk: local weighted prefix into PSUM fp32.
  - cross-chunk carries: a CAR PSUM tile accumulates SFull_c @ e_c (full
    weight columns -> the chunk total replicated on ALL partitions); one
    fp16 snapshot to SBUF per wave (ACT copy).  Because the carry is
    replicated, the carry add folds into the PSUM->SBUF evacuation as a
    DVE tensor_tensor add (engines cannot partition-broadcast, and engine
    APs must start at partition 0/32/64, so a row-per-chunk totals tile is
    not expressible).  The odd chunk of each wave gets the even chunk's
    contribution as one extra SFull matmul accumulated into its PSUM.
  - SFull matrices are built on-device: ones [128,128] * per-partition w
    column (DVE tensor_scalar), no extra DMA.

Everything 16-bit on the wire: x as fp16, weights fp16 scaled by 2^-4 so
they stay in fp16 normal range (h2 * 2^4 applied on host), output fp16.
DMA/core = 4 MB in + 0.5 MB weights + 4 MB out ~ 8.5 MB (~25 us at 340
GB/s).  Host does the transpose/swizzle so all device DMAs are contiguous.
"""

import os
import sys

import numpy as np

sys.path.insert(0, "/opt/trn_rl_repo")

import concourse.bacc as bacc
import concourse.mybir as mybir
import concourse.tile as tile
from concourse import bass_utils

B, M = 8192, 2048
N = M - 1
NCORES = 8
BC = B // NCORES          # 1024 batch cols per core
P = 128                   # SBUF partitions
NCH = M // P              # 16 grid chunks per core
NPAIR = NCH // 2          # 8 chunk pairs (one [128, 2048] tile each)
H2 = (1.0 / N) ** 2
SW = 2.0 ** -4            # weight prescale (keeps w' in fp16 normal range)

_cached_nc = None
LAST_RESULTS = None


def _build_kernel():
    fp32 = mybir.dt.float32
    f16 = mybir.dt.float16
    nc = bacc.Bacc(
        "TRN2",
        target_bir_lowering=False,
        debug=False,
        enable_asserts=False,
        num_devices=NCORES,
    )
    x_d = nc.dram_tensor("x", (BC, 2 * BC), f16, kind="ExternalInput").ap()
    w_d = nc.dram_tensor("w", (P, M), f16, kind="ExternalInput").ap()
    wc_d = nc.dram_tensor("wc", (P, NCH), fp32, kind="ExternalInput").ap()
    o_d = nc.dram_tensor("out", (BC, 2 * BC), f16, kind="ExternalOutput").ap()

    EXP = mybir.ActivationFunctionType.Exp
    ADD = mybir.AluOpType.add

    with tile.TileContext(nc) as tc:
        with (
            tc.tile_pool(name="const", bufs=1) as cpool,
            tc.tile_pool(name="xin", bufs=NPAIR) as xpool,
            tc.tile_pool(name="ee", bufs=3) as epool,
            tc.tile_pool(name="oo", bufs=4) as opool,
            tc.tile_pool(name="cs", bufs=2) as cspool,
            tc.tile_pool(name="ps", bufs=3, space="PSUM") as pspool,
            tc.tile_pool(name="pc", bufs=1, space="PSUM") as carpool,
        ):
            # first x pair in halves so the first exp starts earlier; the
            # weight DMA goes between them on the same in-order ring.
            xts = []
            for p in range(NPAIR):
                xt = xpool.tile([P, 2 * BC], f16, tag="x")
                if p == 0:
                    nc.sync.dma_start(out=xt[:, :BC], in_=x_d[:P, :BC])
                xts.append(xt)
            wt = cpool.tile([P, M], f16, tag="wt")
            nc.sync.dma_start(out=wt, in_=w_d)
            wc = cpool.tile([P, NCH], fp32, tag="wc")
            nc.sync.dma_start(out=wc, in_=wc_d)
            nc.sync.dma_start(out=xts[0][:, BC:], in_=x_d[:P, BC:])
            for p in range(1, NPAIR):
                nc.sync.dma_start(out=xts[p], in_=x_d[p * P : (p + 1) * P, :])

            # SFull matrices: wf[:, cP:(c+1)P] = w' column c broadcast
            onesf = cpool.tile([P, P], f16, tag="onesf")
            nc.vector.memset(onesf, 1.0)
            wf = cpool.tile([P, M], f16, tag="wf")
            for c in range(NCH):
                nc.vector.tensor_scalar_mul(
                    wf[:, c * P : (c + 1) * P],
                    onesf,
                    wc[:, c : c + 1],
                )

            car = carpool.tile([P, BC], fp32, tag="car")
            carsb = {}

            # exp for wave 0 (pair 0) in halves
            ets = {}
            et0 = epool.tile([P, 2 * BC], f16, tag="e")
            nc.scalar.activation(
                out=et0[:, :BC], in_=xts[0][:, :BC], func=EXP, scale=-1.0
            )
            nc.scalar.activation(
                out=et0[:, BC:], in_=xts[0][:, BC:], func=EXP, scale=-1.0
            )
            ets[0] = et0

            for p in range(NPAIR):  # wave == pair: chunks 2p, 2p+1
                if p + 1 < NPAIR:   # prefetch next wave's exp
                    et = epool.tile([P, 2 * BC], f16, tag="e")
                    nc.scalar.activation(
                        out=et, in_=xts[p + 1], func=EXP, scale=-1.0
                    )
                    ets[p + 1] = et
                ep = ets[p]
                ot = opool.tile([P, 2 * BC], f16, tag="o")
                pts = []
                for i in range(2):
                    c = 2 * p + i
                    ec = ep[:, i * BC : (i + 1) * BC]
                    pt = pspool.tile([P, BC], fp32, tag="ps")
                    pts.append(pt)
                    for h in range(2):
                        hs = slice(h * 512, (h + 1) * 512)
                        nc.tensor.matmul(
                            pt[:, hs],
                            lhsT=wt[:, c * P : (c + 1) * P],
                            rhs=ec[:, hs],
                            start=True,
                            stop=(i == 0),
                        )
                    if i == 1:
                        # even chunk's full contribution into odd chunk
                        ea = ep[:, :BC]
                        for h in range(2):
                            hs = slice(h * 512, (h + 1) * 512)
                            nc.tensor.matmul(
                                pt[:, hs],
                                lhsT=wf[:, (c - 1) * P : c * P],
                                rhs=ea[:, hs],
                                start=False,
                                stop=True,
                            )
                    # running cross-wave carry accumulator
                    for h in range(2):
                        hs = slice(h * 512, (h + 1) * 512)
                        nc.tensor.matmul(
                            car[:, hs],
                            lhsT=wf[:, c * P : (c + 1) * P],
                            rhs=ec[:, hs],
                            start=(c == 0),
                            stop=(c == NCH - 1),
                            skip_group_check=True,
                        )
                # snapshot the running carry for the next wave (ACT)
                if p + 1 < NPAIR:
                    cs = cspool.tile([P, BC], f16, tag="cs")
                    nc.scalar.copy(out=cs, in_=car)
                    carsb[p] = cs
                # evacuate with fused carry add (replicated on partitions)
                for i in range(2):
                    dst = ot[:, i * BC : (i + 1) * BC]
                    if p == 0:
                        nc.vector.tensor_copy(out=dst, in_=pts[i])
                    else:
                        nc.vector.tensor_tensor(
                            out=dst, in0=pts[i], in1=carsb[p - 1], op=ADD
                        )
                if p == NPAIR - 1:
                    # last pair: store halves as each evac would land
                    nc.sync.dma_start(
                        out=o_d[p * P : (p + 1) * P, :BC], in_=ot[:, :BC]
                    )
                    nc.sync.dma_start(
                        out=o_d[p * P : (p + 1) * P, BC:], in_=ot[:, BC:]
                    )
                else:
                    nc.sync.dma_start(out=o_d[p * P : (p + 1) * P, :], in_=ot)

    nc.compile()
    return nc


def _get_nc():
    global _cached_nc
    if _cached_nc is None:
        _cached_nc = _build_kernel()
    return _cached_nc


def _make_w(f_rhs: np.ndarray):
    """Triangular weights [128, M] fp16 + per-chunk w columns [128, 16] fp32.

    W[k, 128c + j] = w'_{128c+k} * (k <= j), w' = SW * suffix_sum(f),
    w'_0 halved, w'_{M-1} = 0 (pad).  The fp32 columns feed the on-device
    SFull build (tensor_scalar requires an fp32 scalar operand).
    h2/SW is applied on host afterwards.
    """
    w = np.cumsum(f_rhs[::-1].astype(np.float64))[::-1] * SW
    w[0] *= 0.5
    wq = np.zeros(M, np.float16)
    wq[:N] = w.astype(np.float16)
    mask = np.arange(P)[:, None] <= np.arange(P)[None, :]
    wmat = wq.reshape(NCH, P).T[:, :, None] * mask[:, None, :]
    wcols = np.ascontiguousarray(wq.reshape(NCH, P).T.astype(np.float32))
    return np.ascontiguousarray(wmat.reshape(P, M).astype(np.float16)), wcols


def kernel(x: np.ndarray, f_rhs: np.ndarray) -> np.ndarray:
    assert x.shape == (B, M) and f_rhs.shape == (N,)
    wmat, wcols = _make_w(np.asarray(f_rhs, dtype=np.float32))
    xf = np.asarray(x, dtype=np.float16)
    in_maps = []
    for c in range(NCORES):
        xt = xf[c * BC : (c + 1) * BC].T  # [M, BC] grid-major
        xs = np.ascontiguousarray(
            xt.reshape(NPAIR, 2, P, BC).transpose(0, 2, 1, 3).reshape(BC, 2 * BC)
        )
        in_maps.append({"x": xs, "w": wmat, "wc": wcols})
    nc = _get_nc()
    res = bass_utils.run_bass_kernel_spmd(
        nc,
        in_maps,
        core_ids=list(range(NCORES)),
        trace=bool(int(os.environ.get("KERNEL_TRACE", "0"))),
    )
    global LAST_RESULTS
    LAST_RESULTS = res
    outs = []
    post = np.float32(H2 / SW)
    for c in range(NCORES):
        o = res.results[c]["out"]  # [BC, 2*BC] fp16, swizzled u^T
        ut = (
            np.asarray(o)
            .reshape(NPAIR, P, 2, BC)
            .transpose(0, 2, 1, 3)
            .reshape(M, BC)
        )
        outs.append(ut[:N, :].T.astype(np.float32) * post)
    return np.ascontiguousarray(np.concatenate(outs, axis=0))
